# revision 4
# baseline (speedup 1.0000x reference)
"""KLDiscretLoss joints kernel for TRN2 (8 NeuronCores, Bass/Tile).

Math: for each row (b,j,d) of BINS logits,
  kl_row_sum = sum_bins labels*(log_labels - log_scores)
             = w/St + log(So) - log(St)
  where St = sum(exp(t)), So = sum(exp(o)), w = sum(exp(t)*(t-o)).
(no max-subtraction needed: randn inputs, |x| <~ 6, exp is safe in f32)

Sharding: data-parallel over batch, 32 batches/core -> 1088 rows/core,
tiled as 8x[128,2048] + 1x[64,2048]. Device streams both tensors once
(memory-bound) and emits per-row partial stats; host does the final
per-row combine + batch-mean + sum-over-d + min-over-j in float64.

Schedule notes (from TimelineSim cost model):
- exp on ACT (with fused accum_out row-sum), t-o on Pool/GpSimd,
  mul+reduce on DVE: every engine stays below the ~50us DMA roofline.
- fused tensor_tensor_reduce would save a DVE pass but crashes the NEFF
  on this HW path (NRT_EXEC_UNIT_UNRECOVERABLE) -> mul + reduce.
- the last tiles are bin-halved so the post-last-load dependency chain
  (sub -> mul -> reduce) is half as long; trims ~3us off the tail.
"""

import numpy as np

import concourse.bass as bass
import concourse.tile as tile
from concourse import bacc, mybir
from concourse.bass_utils import run_bass_kernel_spmd

B, J, D, BINS = 256, 17, 2, 2048
NCORES = 8
BS = B // NCORES               # 32 batches per core
ROWS = BS * J * D              # 1088 rows per core
P = 128
NTILES = (ROWS + P - 1) // P   # 9 tiles (8 full + 1 of 64 rows)
SPLIT = {5: 2, 6: 2, 7: 2, 8: 2}  # tail tiles computed in bin-halves
NCOLS = sum(3 * SPLIT.get(i, 1) for i in range(NTILES))
F32 = mybir.dt.float32
Exp = mybir.ActivationFunctionType.Exp
Alu = mybir.AluOpType

_cache = {}


def _build_nc():
    nc = bacc.Bacc(
        "TRN2", target_bir_lowering=False, debug=False, num_devices=NCORES
    )
    o_ap = nc.dram_tensor("o_in", [ROWS, BINS], F32, kind="ExternalInput").ap()
    t_ap = nc.dram_tensor("t_in", [ROWS, BINS], F32, kind="ExternalInput").ap()
    s_ap = nc.dram_tensor("stats", [P, NCOLS], F32, kind="ExternalOutput").ap()

    with tile.TileContext(nc) as tc:
        with (
            tc.tile_pool(name="io", bufs=3) as io,
            tc.tile_pool(name="work", bufs=2) as work,
            tc.tile_pool(name="single", bufs=1) as single,
        ):
            big = single.tile([P, NCOLS], F32)
            nc.vector.memset(big[:], 0.0)
            col = 0
            for i in range(NTILES):
                r0 = i * P
                R = min(P, ROWS - r0)
                nchunk = SPLIT.get(i, 1)
                CS = BINS // nchunk
                for h in range(nchunk):
                    sl = slice(h * CS, (h + 1) * CS)
                    t_t = io.tile([P, BINS], F32, tag="t_t")
                    nc.sync.dma_start(t_t[:R, :CS], t_ap[r0 : r0 + R, sl])
                    o_t = io.tile([P, BINS], F32, tag="o_t")
                    nc.sync.dma_start(o_t[:R, :CS], o_ap[r0 : r0 + R, sl])
                    et = work.tile([P, BINS], F32, tag="et")
                    nc.scalar.activation(
                        et[:R, :CS], t_t[:R, :CS], Exp,
                        accum_out=big[:R, col : col + 1],
                    )
                    eo = work.tile([P, BINS], F32, tag="eo")
                    nc.scalar.activation(
                        eo[:R, :CS], o_t[:R, :CS], Exp,
                        accum_out=big[:R, col + 1 : col + 2],
                    )
                    diff = work.tile([P, BINS], F32, tag="diff")
                    nc.gpsimd.tensor_sub(diff[:R, :CS], t_t[:R, :CS], o_t[:R, :CS])
                    prod = work.tile([P, BINS], F32, tag="prod")
                    nc.vector.tensor_mul(prod[:R, :CS], et[:R, :CS], diff[:R, :CS])
                    nc.vector.tensor_reduce(
                        big[:R, col + 2 : col + 3], prod[:R, :CS],
                        mybir.AxisListType.X, Alu.add,
                    )
                    col += 3
            nc.sync.dma_start(s_ap[:, :], big[:, :])
    nc.compile()
    return nc


def kernel(output, target):
    output = np.ascontiguousarray(output, dtype=np.float32)
    target = np.ascontiguousarray(target, dtype=np.float32)
    assert output.shape == (B, J, D, BINS) and target.shape == (B, J, D, BINS)

    if "nc" not in _cache:
        _cache["nc"] = _build_nc()
    nc = _cache["nc"]

    in_maps = []
    for c in range(NCORES):
        sl = slice(c * BS, (c + 1) * BS)
        in_maps.append(
            {
                "o_in": output[sl].reshape(ROWS, BINS),
                "t_in": target[sl].reshape(ROWS, BINS),
            }
        )

    res = run_bass_kernel_spmd(nc, in_maps, list(range(NCORES)))
    _cache["last_results"] = res

    # host-side decode + final reduction (float64)
    per_row = np.empty((NCORES, ROWS), dtype=np.float64)
    for c in range(NCORES):
        st = res.results[c]["stats"].astype(np.float64)  # [P, NCOLS]
        St = np.zeros((NTILES, P))
        So = np.zeros((NTILES, P))
        w = np.zeros((NTILES, P))
        col = 0
        for i in range(NTILES):
            for _h in range(SPLIT.get(i, 1)):
                St[i] += st[:, col]
                So[i] += st[:, col + 1]
                w[i] += st[:, col + 2]
                col += 3
        St = St.reshape(-1)[:ROWS]
        So = So.reshape(-1)[:ROWS]
        w = w.reshape(-1)[:ROWS]
        per_row[c] = w / St + np.log(So) - np.log(St)

    per_row = per_row.reshape(B, J * D) / BINS          # per_bd, mean over bins
    per_jd = per_row.mean(axis=0)                        # [J*D]
    loss = per_jd.reshape(J, D).sum(axis=1)              # [J]
    return np.float32(loss.min())
